# revision 3
# baseline (speedup 1.0000x reference)
"""MoE (top-2 of 16 experts, SwiGLU MLP) kernel for 8 Trainium2 NeuronCores.

Strategy (expert-parallel, per sharding hint):
  - Host: router (x @ w_gate -> softmax -> top-2) in float64; tokens
    gathered per expert. Experts ranked by token count: 8 largest ->
    core slot 0, 8 smallest -> slot 1; each slot gets a uniform
    capacity = its max count rounded to 8.
  - Everything streamed to the device is bf16 and pre-laid-out on host
    in the exact SBUF tile layout, so every DMA descriptor moves a
    contiguous multi-KB run. All weights + tokens fit in SBUF at bf16.
  - Device (SPMD over 8 cores, 2 experts/core): per expert
        ht = silu(W1e.T @ Xt) * (W2e.T @ Xt)     [feature-major]
        yt = WCe.T @ ht                           (gate applied on host)
    bf16 matmuls accumulate fp32 in PSUM.
  - Host: out[tok] += gate * yt  (scatter-add, fp32).

Timing model (trace-driven):
  - ~7.3us fixed NEFF preamble (start fence + iram load) before ANY
    program instruction runs; first DMA byte lands ~8us.
  - PE clock is HAM-gated: 1.2 GHz until ~3.4us of sustained matmul
    activity, then 2.4 GHz.  Small N=128 filler matmuls on a scratch
    tile start the activity window at ~7.3us and bridge every head
    DMA-wait gap so the clock never re-throttles.
  - Weights stream on the Activation HWDGE queue, tokens + y-stores on
    the SP queue, both in just-in-time program order.  The third token
    chunk of the big expert is deferred to a post-sweep so the first
    h-panels only need chunk0+chunk1.
  - Tail: the last d-row of the last expert stores per-chunk (final
    piece split in half across both DMA queues) so almost nothing
    drains after the last matmul.
"""

import contextlib
import ctypes
import os
import sys
import types

sys.path.insert(0, "/opt/trn_rl_repo")

import ml_dtypes
import numpy as np

import concourse.bass as bass
import concourse.mybir as mybir
import concourse.tile as tile

EMB = 1024
HID = 1024
E = 16
TOPK = 2
NCORES = 8
EPC = E // NCORES  # experts per core
P = 128
KT = EMB // P  # contraction tiles (8)
HT = HID // P  # hidden row-blocks (8)
BF16 = ml_dtypes.bfloat16

# --- head tunables (filler = N=128 scratch matmul, 107ns cold / 53ns warm) ---
F_HEAD = 16          # fillers before the first real matmul
H0_FILLS = (1, 2, 1, 2, 3, 0)  # fillers after each k-half group of h0
H1_FILLS = 2         # fillers at entry of h1 unit
H2_FILLS = 1         # fillers at entry of h2 unit


def _install_profile_shim():
    """Register the axon NTFF profiling hook (missing antenv.axon_hooks in
    this image) so run_bass_kernel_spmd(trace=True) can measure HW time."""
    if "antenv.axon_hooks" in sys.modules:
        return
    try:
        lib = ctypes.CDLL("/opt/axon/libaxon_pjrt.so")
        lib.axon_start_nrt_profile.argtypes = [
            ctypes.POINTER(ctypes.c_int64),
            ctypes.c_size_t,
        ]
        lib.axon_start_nrt_profile.restype = ctypes.c_int64
        lib.axon_stop_nrt_profile.argtypes = [ctypes.c_char_p]
        lib.axon_stop_nrt_profile.restype = ctypes.c_int64
    except Exception:
        return

    @contextlib.contextmanager
    def _hook(output_dir, device_ids):
        import jax

        jax.devices()
        ids = (
            (ctypes.c_int64 * len(device_ids))(*device_ids) if device_ids else None
        )
        rc = lib.axon_start_nrt_profile(ids, len(device_ids) if device_ids else 0)
        if rc != 0:
            raise RuntimeError(f"axon_start_nrt_profile rc={rc}")
        try:
            yield
        finally:
            n = lib.axon_stop_nrt_profile(str(output_dir).encode())
            print(f"profile: {n} file(s) written to {output_dir}")

    mod = types.ModuleType("antenv.axon_hooks")
    mod.get_axon_ntff_profile_hook = lambda: _hook
    mod.set_axon_ntff_profile_hook = lambda h: None
    sys.modules["antenv.axon_hooks"] = mod


def _split_multi_waits(nc):
    """This container's walrus only encodes one sem wait per CTRL-class
    instruction; hoist extra waits onto dedicated single-wait NoOps."""
    idx = 0
    for fn in nc.m.functions:
        for bb in fn.blocks:
            new = []
            for inst in bb.instructions:
                si = inst.sync_info
                if si is not None and len(si.on_wait) > 1:
                    waits = list(si.on_wait)
                    for w in waits[:-1]:
                        c = mybir.InstNoOp(name=f"wsplit-{idx}", ins=[], outs=[])
                        idx += 1
                        c.engine = inst.engine
                        c.sync_info = mybir.SyncInfo(on_wait=[w], on_update=[])
                        new.append(c)
                    si.on_wait = [waits[-1]]
                new.append(inst)
            bb.instructions = new


def _token_chunks(C):
    """Split C into chunks <=512, remainder first (the remainder chunk is
    the first thing the PE touches, so keep it small for a fast start)."""
    rem = C % 512
    return ([rem] if rem else []) + [512] * (C // 512)


def _build_bass(caps):
    F32 = mybir.dt.float32
    BF = mybir.dt.bfloat16

    nc = bass.Bass()
    chunks = [_token_chunks(caps[0]), _token_chunks(caps[1])]
    offs = []
    for e in range(EPC):
        o, lst = 0, []
        for cs in chunks[e]:
            lst.append(o)
            o += cs
        offs.append(lst)

    xt_d, w1_d, w2_d, wc_d, yt_d = [], [], [], [], []
    for e in range(EPC):
        C = caps[e]
        # all pre-laid-out on host: per-partition rows are contiguous.
        # xt is chunk-major: [chunk0: k0..k7 x cs0][chunk1: ...].
        xt_d.append(nc.declare_dram_parameter(f"xt{e}", [P, KT * C], BF, isOutput=False))
        w1_d.append(nc.declare_dram_parameter(f"w1_{e}", [P, HT, KT, P], BF, isOutput=False))
        w2_d.append(nc.declare_dram_parameter(f"w2_{e}", [P, HT, KT, P], BF, isOutput=False))
        wc_d.append(nc.declare_dram_parameter(f"wc_{e}", [P, HT, KT, P], BF, isOutput=False))
        yt_d.append(nc.declare_dram_parameter(f"yt{e}", [P, HT, C], BF, isOutput=True))

    with tile.TileContext(nc) as tc:
        with (
            tc.tile_pool(name="xt", bufs=1) as xt_pool,
            tc.tile_pool(name="ht", bufs=1) as ht_pool,
            tc.tile_pool(name="w", bufs=1) as w_pool,
            tc.tile_pool(name="warm", bufs=1) as warm_pool,
            tc.tile_pool(name="s", bufs=3) as s_pool,
            tc.tile_pool(name="y", bufs=2) as y_pool,
            tc.tile_pool(name="psA", bufs=4, space="PSUM") as psA,
            tc.tile_pool(name="psB", bufs=3, space="PSUM") as psB,
            tc.tile_pool(name="psF", bufs=1, space="PSUM") as psF,
        ):
            # --- long-lived SBUF tiles (everything fits at bf16) ---
            xt_sb = [
                xt_pool.tile([P, KT * caps[e]], BF, tag=f"xt{e}", name=f"xt_sb{e}")
                for e in range(EPC)
            ]
            ht_sb = [
                ht_pool.tile([P, HT, caps[e]], BF, tag=f"ht{e}", name=f"ht_sb{e}")
                for e in range(EPC)
            ]
            w1_sb = [
                w_pool.tile([P, HT, KT, P], BF, tag=f"w1{e}", name=f"w1_sb{e}")
                for e in range(EPC)
            ]
            w2_sb = [
                w_pool.tile([P, HT, KT, P], BF, tag=f"w2{e}", name=f"w2_sb{e}")
                for e in range(EPC)
            ]
            wc_sb = [
                w_pool.tile([P, HT, KT, P], BF, tag=f"wc{e}", name=f"wc_sb{e}")
                for e in range(EPC)
            ]

            # --- HAM warm-up scratch: never DMA'd, contents irrelevant
            # (results land in the never-read psF bank).  Memset rides
            # gpsimd, which is idle and ready before the tensor engine,
            # so the first fillers aren't stalled behind init work. ---
            warm_sb = warm_pool.tile([P, P], BF, tag="warm")
            nc.gpsimd.memset(warm_sb[:], 0.0)
            warm_ps = psF.tile([P, P], F32, tag="fill")

            def filler(n):
                for _ in range(n):
                    nc.tensor.matmul(
                        warm_ps[:], warm_sb[:], warm_sb[:], start=True, stop=True
                    )

            # --- DMA helpers.  Weights ride the Activation HWDGE queue,
            # tokens + y-stores the SP queue; program order per engine is
            # the wire order, so dispatches are emitted just-in-time. ---
            def wdma(sb_l, d_l, e, h, k0=0, k1=KT):
                nc.scalar.dma_start(sb_l[e][:, h, k0:k1], d_l[e][:, h, k0:k1])

            def wdma_full(sb_l, d_l, e):
                nc.scalar.dma_start(sb_l[e][:], d_l[e][:])

            def xdma(e, ci):
                off, cs = offs[e][ci], chunks[e][ci]
                nc.sync.dma_start(
                    xt_sb[e][:, off * KT : (off + cs) * KT],
                    xt_d[e][:, off * KT : (off + cs) * KT],
                )

            # head wave: h0 weight k-halves (small pieces -> early first
            # matmul), first two token chunks whole (big descriptors).
            wdma(w1_sb, w1_d, 0, 0, 0, 4)
            wdma(w1_sb, w1_d, 0, 0, 4, 8)
            wdma(w2_sb, w2_d, 0, 0, 0, 4)
            wdma(w2_sb, w2_d, 0, 0, 4, 8)
            xdma(0, 0)
            xdma(0, 1)
            for ci in range(2, len(chunks[0])):
                xdma(0, ci)

            def xt_mv(e, off, k, cs):
                # moving operand: [128, cs] slice of chunk at offset off
                base = off * KT + k * cs
                return xt_sb[e][:, base : base + cs]

            def a_part(e, h, off, cs, wsb, ps, k0, k1):
                for k in range(k0, k1):
                    nc.tensor.matmul(
                        ps[:],
                        wsb[e][:, h, k],
                        xt_mv(e, off, k, cs),
                        start=(k == 0),
                        stop=(k == KT - 1),
                    )

            def a_evict(e, h, off, cs, ps1, ps2):
                s_sb = s_pool.tile([P, 512], F32, tag="s")
                nc.scalar.activation(
                    s_sb[:, :cs], ps1[:], mybir.ActivationFunctionType.Silu
                )
                nc.vector.tensor_mul(
                    ht_sb[e][:, h, off : off + cs], s_sb[:, :cs], ps2[:]
                )

            def a_unit(e, h, cis):
                for ci in cis:
                    off, cs = offs[e][ci], chunks[e][ci]
                    ps1 = psA.tile([P, cs], F32, tag="ps")
                    ps2 = psA.tile([P, cs], F32, tag="ps")
                    a_part(e, h, off, cs, w1_sb, ps1, 0, KT)
                    a_part(e, h, off, cs, w2_sb, ps2, 0, KT)
                    a_evict(e, h, off, cs, ps1, ps2)

            # ---- tensor-engine head: fillers bridge until data lands ----
            filler(F_HEAD)

            ofa, csa = offs[0][0], chunks[0][0]
            ofb, csb = offs[0][1], chunks[0][1]
            p1a = psA.tile([P, csa], F32, tag="ps")
            p2a = psA.tile([P, csa], F32, tag="ps")
            p1b = psA.tile([P, csb], F32, tag="ps")
            p2b = psA.tile([P, csb], F32, tag="ps")
            a_part(0, 0, ofa, csa, w1_sb, p1a, 0, 4)
            filler(H0_FILLS[0])
            a_part(0, 0, ofa, csa, w1_sb, p1a, 4, 8)
            filler(H0_FILLS[1])
            a_part(0, 0, ofa, csa, w2_sb, p2a, 0, 4)
            filler(H0_FILLS[2])
            a_part(0, 0, ofa, csa, w2_sb, p2a, 4, 8)
            filler(H0_FILLS[3])
            a_part(0, 0, ofb, csb, w1_sb, p1b, 0, KT)
            filler(H0_FILLS[4])
            a_part(0, 0, ofb, csb, w2_sb, p2b, 0, KT)
            filler(H0_FILLS[5])
            # h1 weights dispatch before the evicts claim the scalar queue
            wdma(w1_sb, w1_d, 0, 1)
            wdma(w2_sb, w2_d, 0, 1)
            a_evict(0, 0, ofa, csa, p1a, p2a)
            a_evict(0, 0, ofb, csb, p1b, p2b)

            # ---- e0 phase A: h1..h7 on chunks {0,1}; chunk 2+ deferred
            # to a post-sweep so early h-panels need minimal data. ----
            head_cis = [0, 1]
            defer_cis = list(range(2, len(chunks[0])))
            for h in range(1, HT):
                if h == 1:
                    filler(H1_FILLS)
                elif h == 2:
                    filler(H2_FILLS)
                if h + 1 < HT:
                    wdma(w1_sb, w1_d, 0, h + 1)
                    wdma(w2_sb, w2_d, 0, h + 1)
                else:
                    wdma_full(wc_sb, wc_d, 0)
                if h == 3:
                    for ci in range(len(chunks[1])):
                        xdma(1, ci)
                a_unit(0, h, head_cis)
            if defer_cis:
                for h in range(HT):
                    if h == 0:
                        wdma_full(w1_sb, w1_d, 1)
                    elif h == 2:
                        wdma_full(w2_sb, w2_d, 1)
                    elif h == 4:
                        wdma_full(wc_sb, wc_d, 1)
                    a_unit(0, h, defer_cis)
            else:
                wdma_full(w1_sb, w1_d, 1)
                wdma_full(w2_sb, w2_d, 1)
                wdma_full(wc_sb, wc_d, 1)

            # ---- phase B: yt = WC.T @ ht  (gate applied on host) ----
            def phase_b(e, last):
                C = caps[e]
                for d in range(HT):
                    y_sb = y_pool.tile([P, C], BF, tag="y")
                    if not (last and d == HT - 1):
                        for ci, cs in enumerate(chunks[e]):
                            off = offs[e][ci]
                            psy = psB.tile([P, cs], F32, tag="psy")
                            for h in range(HT):
                                nc.tensor.matmul(
                                    psy[:],
                                    wc_sb[e][:, d, h],
                                    ht_sb[e][:, h, off : off + cs],
                                    start=(h == 0),
                                    stop=(h == HT - 1),
                                )
                            nc.scalar.copy(y_sb[:, off : off + cs], psy[:])
                        nc.sync.dma_start(yt_d[e][:, d], y_sb[:])
                    else:
                        # tail: big chunks first with per-chunk stores,
                        # then the small chunk split across both queues.
                        order = list(range(len(chunks[e])))[::-1]
                        for ci in order[:-1]:
                            off, cs = offs[e][ci], chunks[e][ci]
                            psy = psB.tile([P, cs], F32, tag="psy")
                            for h in range(HT):
                                nc.tensor.matmul(
                                    psy[:],
                                    wc_sb[e][:, d, h],
                                    ht_sb[e][:, h, off : off + cs],
                                    start=(h == 0),
                                    stop=(h == HT - 1),
                                )
                            nc.scalar.copy(y_sb[:, off : off + cs], psy[:])
                            nc.sync.dma_start(
                                yt_d[e][:, d, off : off + cs],
                                y_sb[:, off : off + cs],
                            )
                        ci = order[-1]
                        off, cs = offs[e][ci], chunks[e][ci]
                        psy = psB.tile([P, cs], F32, tag="psy")
                        for h in range(HT):
                            nc.tensor.matmul(
                                psy[:],
                                wc_sb[e][:, d, h],
                                ht_sb[e][:, h, off : off + cs],
                                start=(h == 0),
                                stop=(h == HT - 1),
                            )
                        ha = (cs // 2) // 4 * 4
                        nc.scalar.copy(y_sb[:, off : off + ha], psy[:, :ha])
                        nc.scalar.dma_start(
                            yt_d[e][:, d, off : off + ha], y_sb[:, off : off + ha]
                        )
                        nc.scalar.copy(
                            y_sb[:, off + ha : off + cs], psy[:, ha:cs]
                        )
                        nc.sync.dma_start(
                            yt_d[e][:, d, off + ha : off + cs],
                            y_sb[:, off + ha : off + cs],
                        )

            phase_b(0, last=False)

            # ---- e1: weights/tokens long resident; straight stream ----
            for h in range(HT):
                a_unit(1, h, range(len(chunks[1])))
            phase_b(1, last=True)

    _split_multi_waits(nc)
    return nc


def kernel(x, w_gate, w1, w2, wc):
    trace = bool(int(os.environ.get("BASS_MOE_TRACE", "0")))
    if trace:
        _install_profile_shim()

    import concourse.bass_utils as bass_utils

    bass_utils.upload_artifacts = lambda tmpdir: f"local://{tmpdir}"

    x = np.asarray(x, dtype=np.float32)
    w_gate = np.asarray(w_gate, dtype=np.float32)
    w1 = np.asarray(w1, dtype=np.float32)
    w2 = np.asarray(w2, dtype=np.float32)
    wc = np.asarray(wc, dtype=np.float32)

    b, s, d = x.shape
    xf = x.reshape(-1, d)
    n = xf.shape[0]

    # ---- Router on host (float64: stable ranking + gate values) ----
    logits = xf.astype(np.float64) @ w_gate.astype(np.float64)
    mx = logits.max(axis=1, keepdims=True)
    p = np.exp(logits - mx)
    p /= p.sum(axis=1, keepdims=True)
    top = np.argpartition(-logits, TOPK, axis=1)[:, :TOPK]  # top-2 ids (unordered)

    sel_tok = []  # per expert: token indices
    sel_gate = []  # per expert: gate values
    flat_e = top.ravel()
    flat_t = np.repeat(np.arange(n), TOPK)
    order = np.argsort(flat_e, kind="stable")
    se, st = flat_e[order], flat_t[order]
    bounds = np.searchsorted(se, np.arange(E + 1))
    counts = np.diff(bounds)
    for e in range(E):
        toks = st[bounds[e] : bounds[e + 1]]
        sel_tok.append(toks)
        sel_gate.append(p[toks, e])

    # ---- Slot assignment: biggest experts in slot 0, smallest in slot 1,
    # so each slot's uniform capacity hugs its experts' actual counts ----
    rank = np.argsort(-counts, kind="stable")
    slot_experts = [
        [int(rank[core + j * NCORES]) for j in range(EPC)] for core in range(NCORES)
    ]
    caps = []
    for j in range(EPC):
        cmax = max(counts[slot_experts[core][j]] for core in range(NCORES))
        caps.append(max(16, int(-(-cmax // 8) * 8)))

    # ---- Per-core input maps: bf16, pre-laid-out in SBUF tile order ----
    xf_bf = xf.astype(BF16)
    # weight layout [p, h, k, q] = w[k*128+p, h*128+q] (contiguous per
    # partition-row => 1 DMA descriptor per partition)
    def wlayout(w):
        return np.ascontiguousarray(
            w.astype(BF16).reshape(KT, P, HT, P).transpose(1, 2, 0, 3)
        )

    chunk_lists = [_token_chunks(caps[0]), _token_chunks(caps[1])]
    in_maps = []
    for core in range(NCORES):
        m = {}
        for j in range(EPC):
            e = slot_experts[core][j]
            C = caps[j]
            toks = sel_tok[e]
            # chunk-major: [chunk: [k: [c]]] flattened to [P, KT*C]
            xe = np.zeros((P, KT, C), dtype=BF16)
            xg = xf_bf[toks].T.reshape(KT, P, len(toks))
            xe[:, :, : len(toks)] = xg.transpose(1, 0, 2)
            xt = np.empty((P, KT * C), dtype=BF16)
            c0 = 0
            for cs in chunk_lists[j]:
                xt[:, c0 * KT : (c0 + cs) * KT] = xe[:, :, c0 : c0 + cs].reshape(
                    P, KT * cs
                )
                c0 += cs
            m[f"xt{j}"] = xt
            m[f"w1_{j}"] = wlayout(w1[e])
            m[f"w2_{j}"] = wlayout(w2[e])
            m[f"wc_{j}"] = wlayout(wc[e])
        in_maps.append(m)

    nc = _build_bass(caps)
    res = bass_utils.run_bass_kernel_spmd(
        nc, in_maps, list(range(NCORES)), trace=trace
    )
    if trace:
        kernel.last_exec_time_ns = res.exec_time_ns
        kernel.last_trace = (
            res.instructions_and_trace[1] if res.instructions_and_trace else None
        )

    # ---- Scatter-add back to token order, applying gates on host ----
    out = np.zeros((n, d), dtype=np.float64)
    for core in range(NCORES):
        for j in range(EPC):
            e = slot_experts[core][j]
            toks = sel_tok[e]
            yt = res.results[core][f"yt{j}"]  # [P, HT, C] bf16
            yv = (
                yt[:, :, : len(toks)]
                .transpose(1, 0, 2)
                .reshape(EMB, len(toks))
                .astype(np.float64)
            )
            out[toks] += sel_gate[e][:, None] * yv.T
    return out.astype(np.float32).reshape(b, s, d)


# revision 7
# speedup vs baseline: 1.0261x; 1.0261x over previous
"""MoE (top-2 of 16 experts, SwiGLU MLP) kernel for 8 Trainium2 NeuronCores.

Strategy (expert-parallel, per sharding hint):
  - Host: router (x @ w_gate -> softmax -> top-2) in float64; tokens
    gathered per expert. Experts ranked by token count: 8 largest ->
    core slot 0, 8 smallest -> slot 1; each slot gets a uniform
    capacity = its max count rounded to 8.
  - Everything streamed to the device is bf16 and pre-laid-out on host
    in the exact SBUF tile layout, so every DMA descriptor moves a
    contiguous multi-KB run. All weights + tokens fit in SBUF at bf16.
  - Device (SPMD over 8 cores, 2 experts/core): per expert
        ht = silu(W1e.T @ Xt) * (W2e.T @ Xt)     [feature-major]
        yt = WCe.T @ ht                           (gate applied on host)
    bf16 matmuls accumulate fp32 in PSUM.
  - Host: out[tok] += gate * yt  (scatter-add, fp32).

Timing model (trace-driven):
  - ~7.3us fixed NEFF preamble (start fence + iram load) before ANY
    program instruction runs; first DMA byte lands ~8us.
  - PE clock is HAM-gated: 1.2 GHz until ~3.4us of sustained matmul
    activity, then 2.4 GHz.  Small N=128 filler matmuls on a scratch
    tile start the activity window at ~7.3us and bridge every head
    DMA-wait gap so the clock never re-throttles.
  - Weights stream on the Activation HWDGE queue, tokens + y-stores on
    the SP queue, both in just-in-time program order.  The third token
    chunk of the big expert is deferred to a post-sweep so the first
    h-panels only need chunk0+chunk1.
  - Tail: the last d-row of the last expert stores per-chunk (final
    piece split in half across both DMA queues) so almost nothing
    drains after the last matmul.
"""

import contextlib
import ctypes
import os
import sys
import types

sys.path.insert(0, "/opt/trn_rl_repo")

import ml_dtypes
import numpy as np

import concourse.bass as bass
import concourse.mybir as mybir
import concourse.tile as tile

EMB = 1024
HID = 1024
E = 16
TOPK = 2
NCORES = 8
EPC = E // NCORES  # experts per core
P = 128
KT = EMB // P  # contraction tiles (8)
HT = HID // P  # hidden row-blocks (8)
BF16 = ml_dtypes.bfloat16

# --- head tunables (filler = N=128 scratch matmul, 107ns cold / 53ns warm) ---
F_HEAD = 22          # fillers before the first real matmul (cover ~7.3->9.7us)
F_W2 = 4             # fillers between the w1-c0 and w2-c0 groups
F_C1 = 36            # fillers bridging until chunk1 lands (~11->13.4us)


def _install_profile_shim():
    """Register the axon NTFF profiling hook (missing antenv.axon_hooks in
    this image) so run_bass_kernel_spmd(trace=True) can measure HW time."""
    if "antenv.axon_hooks" in sys.modules:
        return
    try:
        lib = ctypes.CDLL("/opt/axon/libaxon_pjrt.so")
        lib.axon_start_nrt_profile.argtypes = [
            ctypes.POINTER(ctypes.c_int64),
            ctypes.c_size_t,
        ]
        lib.axon_start_nrt_profile.restype = ctypes.c_int64
        lib.axon_stop_nrt_profile.argtypes = [ctypes.c_char_p]
        lib.axon_stop_nrt_profile.restype = ctypes.c_int64
    except Exception:
        return

    @contextlib.contextmanager
    def _hook(output_dir, device_ids):
        import jax

        jax.devices()
        ids = (
            (ctypes.c_int64 * len(device_ids))(*device_ids) if device_ids else None
        )
        rc = lib.axon_start_nrt_profile(ids, len(device_ids) if device_ids else 0)
        if rc != 0:
            raise RuntimeError(f"axon_start_nrt_profile rc={rc}")
        try:
            yield
        finally:
            n = lib.axon_stop_nrt_profile(str(output_dir).encode())
            print(f"profile: {n} file(s) written to {output_dir}")

    mod = types.ModuleType("antenv.axon_hooks")
    mod.get_axon_ntff_profile_hook = lambda: _hook
    mod.set_axon_ntff_profile_hook = lambda h: None
    sys.modules["antenv.axon_hooks"] = mod


def _split_multi_waits(nc):
    """This container's walrus only encodes one sem wait per CTRL-class
    instruction; hoist extra waits onto dedicated single-wait NoOps."""
    idx = 0
    for fn in nc.m.functions:
        for bb in fn.blocks:
            new = []
            for inst in bb.instructions:
                si = inst.sync_info
                if si is not None and len(si.on_wait) > 1:
                    waits = list(si.on_wait)
                    for w in waits[:-1]:
                        c = mybir.InstNoOp(name=f"wsplit-{idx}", ins=[], outs=[])
                        idx += 1
                        c.engine = inst.engine
                        c.sync_info = mybir.SyncInfo(on_wait=[w], on_update=[])
                        new.append(c)
                    si.on_wait = [waits[-1]]
                new.append(inst)
            bb.instructions = new


def _token_chunks(C):
    """Split C into chunks <=512, remainder first (the remainder chunk is
    the first thing the PE touches, so keep it small for a fast start)."""
    rem = C % 512
    return ([rem] if rem else []) + [512] * (C // 512)


def _build_bass(caps):
    F32 = mybir.dt.float32
    BF = mybir.dt.bfloat16

    nc = bass.Bass()
    chunks = [_token_chunks(caps[0]), _token_chunks(caps[1])]
    offs = []
    for e in range(EPC):
        o, lst = 0, []
        for cs in chunks[e]:
            lst.append(o)
            o += cs
        offs.append(lst)

    xt_d, w1_d, w2_d, wc_d, yt_d = [], [], [], [], []
    for e in range(EPC):
        C = caps[e]
        # all pre-laid-out on host: per-partition rows are contiguous.
        # xt is chunk-major: [chunk0: k0..k7 x cs0][chunk1: ...].
        xt_d.append(nc.declare_dram_parameter(f"xt{e}", [P, KT * C], BF, isOutput=False))
        w1_d.append(nc.declare_dram_parameter(f"w1_{e}", [P, HT, KT, P], BF, isOutput=False))
        w2_d.append(nc.declare_dram_parameter(f"w2_{e}", [P, HT, KT, P], BF, isOutput=False))
        wc_d.append(nc.declare_dram_parameter(f"wc_{e}", [P, HT, KT, P], BF, isOutput=False))
        yt_d.append(nc.declare_dram_parameter(f"yt{e}", [P, HT, C], BF, isOutput=True))

    with tile.TileContext(nc) as tc:
        with (
            tc.tile_pool(name="xt", bufs=1) as xt_pool,
            tc.tile_pool(name="ht", bufs=1) as ht_pool,
            tc.tile_pool(name="w", bufs=1) as w_pool,
            tc.tile_pool(name="warm", bufs=1) as warm_pool,
            tc.tile_pool(name="s", bufs=3) as s_pool,
            tc.tile_pool(name="y", bufs=2) as y_pool,
            tc.tile_pool(name="psA", bufs=4, space="PSUM") as psA,
            tc.tile_pool(name="psB", bufs=3, space="PSUM") as psB,
            tc.tile_pool(name="psF", bufs=1, space="PSUM") as psF,
        ):
            # --- long-lived SBUF tiles (everything fits at bf16) ---
            xt_sb = [
                xt_pool.tile([P, KT * caps[e]], BF, tag=f"xt{e}", name=f"xt_sb{e}")
                for e in range(EPC)
            ]
            ht_sb = [
                ht_pool.tile([P, HT, caps[e]], BF, tag=f"ht{e}", name=f"ht_sb{e}")
                for e in range(EPC)
            ]
            w1_sb = [
                w_pool.tile([P, HT, KT, P], BF, tag=f"w1{e}", name=f"w1_sb{e}")
                for e in range(EPC)
            ]
            w2_sb = [
                w_pool.tile([P, HT, KT, P], BF, tag=f"w2{e}", name=f"w2_sb{e}")
                for e in range(EPC)
            ]
            wc_sb = [
                w_pool.tile([P, HT, KT, P], BF, tag=f"wc{e}", name=f"wc_sb{e}")
                for e in range(EPC)
            ]

            # --- HAM warm-up scratch: never DMA'd, contents irrelevant
            # (results land in the never-read psF bank).  Memset rides
            # gpsimd, which is idle and ready before the tensor engine,
            # so the first fillers aren't stalled behind init work. ---
            warm_sb = warm_pool.tile([P, P], BF, tag="warm")
            nc.gpsimd.memset(warm_sb[:], 0.0)
            warm_ps = psF.tile([P, P], F32, tag="fill")

            def filler(n):
                for _ in range(n):
                    nc.tensor.matmul(
                        warm_ps[:], warm_sb[:], warm_sb[:], start=True, stop=True
                    )

            # --- DMA schedule.  ALL inputs ride the single SP HWDGE
            # queue, dispatched up-front in exact consumption order: the
            # wire serves one queue FIFO, so arrival order is guaranteed
            # and nothing (like a later expert's tokens) can steal
            # bandwidth from the head.  y-stores join the same queue
            # later (inputs are long done by then). ---
            def wdma(sb_l, d_l, e, h):
                nc.sync.dma_start(sb_l[e][:, h], d_l[e][:, h])

            def wdma_full(sb_l, d_l, e):
                nc.sync.dma_start(sb_l[e][:], d_l[e][:])

            def xdma(e, ci):
                off, cs = offs[e][ci], chunks[e][ci]
                nc.sync.dma_start(
                    xt_sb[e][:, off * KT : (off + cs) * KT],
                    xt_d[e][:, off * KT : (off + cs) * KT],
                )

            xdma(0, 0)                      # chunk0 (small remainder)
            wdma(w1_sb, w1_d, 0, 0)         # h0 weight panels
            wdma(w2_sb, w2_d, 0, 0)
            xdma(0, 1)                      # chunk1
            for h in range(1, 5):           # h1..h4 panels
                wdma(w1_sb, w1_d, 0, h)
                wdma(w2_sb, w2_d, 0, h)
            for ci in range(2, len(chunks[0])):
                xdma(0, ci)                 # deferred chunk(s)
            for h in range(5, HT):          # h5..h7 panels
                wdma(w1_sb, w1_d, 0, h)
                wdma(w2_sb, w2_d, 0, h)
            wdma_full(wc_sb, wc_d, 0)
            for ci in range(len(chunks[1])):
                xdma(1, ci)
            wdma_full(w1_sb, w1_d, 1)
            wdma_full(w2_sb, w2_d, 1)
            wdma_full(wc_sb, wc_d, 1)

            def xt_mv(e, off, k, cs):
                # moving operand: [128, cs] slice of chunk at offset off
                base = off * KT + k * cs
                return xt_sb[e][:, base : base + cs]

            def a_part(e, h, off, cs, wsb, ps, k0, k1):
                for k in range(k0, k1):
                    nc.tensor.matmul(
                        ps[:],
                        wsb[e][:, h, k],
                        xt_mv(e, off, k, cs),
                        start=(k == 0),
                        stop=(k == KT - 1),
                    )

            def a_evict(e, h, off, cs, ps1, ps2):
                s_sb = s_pool.tile([P, 512], F32, tag="s")
                nc.scalar.activation(
                    s_sb[:, :cs], ps1[:], mybir.ActivationFunctionType.Silu
                )
                nc.vector.tensor_mul(
                    ht_sb[e][:, h, off : off + cs], s_sb[:, :cs], ps2[:]
                )

            def a_unit(e, h, cis):
                for ci in cis:
                    off, cs = offs[e][ci], chunks[e][ci]
                    ps1 = psA.tile([P, cs], F32, tag="ps")
                    ps2 = psA.tile([P, cs], F32, tag="ps")
                    a_part(e, h, off, cs, w1_sb, ps1, 0, KT)
                    a_part(e, h, off, cs, w2_sb, ps2, 0, KT)
                    a_evict(e, h, off, cs, ps1, ps2)

            # ---- tensor-engine head: fillers bridge until data lands ----
            filler(F_HEAD)

            ofa, csa = offs[0][0], chunks[0][0]
            ofb, csb = offs[0][1], chunks[0][1]
            p1a = psA.tile([P, csa], F32, tag="ps")
            p2a = psA.tile([P, csa], F32, tag="ps")
            p1b = psA.tile([P, csb], F32, tag="ps")
            p2b = psA.tile([P, csb], F32, tag="ps")
            a_part(0, 0, ofa, csa, w1_sb, p1a, 0, KT)
            filler(F_W2)
            a_part(0, 0, ofa, csa, w2_sb, p2a, 0, KT)
            filler(F_C1)
            a_part(0, 0, ofb, csb, w1_sb, p1b, 0, KT)
            a_part(0, 0, ofb, csb, w2_sb, p2b, 0, KT)
            a_evict(0, 0, ofa, csa, p1a, p2a)
            a_evict(0, 0, ofb, csb, p1b, p2b)

            # ---- e0 phase A: h1..h7 on chunks {0,1}; chunk 2+ deferred
            # to a post-sweep so early h-panels need minimal data. ----
            head_cis = [0, 1]
            defer_cis = list(range(2, len(chunks[0])))
            for h in range(1, HT):
                a_unit(0, h, head_cis)
            for h in range(HT):
                a_unit(0, h, defer_cis)

            # ---- phase B: yt = WC.T @ ht  (gate applied on host) ----
            def b_group(e, d, ci, psy):
                off, cs = offs[e][ci], chunks[e][ci]
                for h in range(HT):
                    nc.tensor.matmul(
                        psy[:],
                        wc_sb[e][:, d, h],
                        ht_sb[e][:, h, off : off + cs],
                        start=(h == 0),
                        stop=(h == HT - 1),
                    )

            def phase_b(e, last):
                C = caps[e]
                for d in range(HT):
                    y_sb = y_pool.tile([P, C], BF, tag="y")
                    if not (last and d >= HT - 2):
                        for ci, cs in enumerate(chunks[e]):
                            off = offs[e][ci]
                            psy = psB.tile([P, cs], F32, tag="psy")
                            b_group(e, d, ci, psy)
                            nc.scalar.copy(y_sb[:, off : off + cs], psy[:])
                        nc.sync.dma_start(yt_d[e][:, d], y_sb[:])
                    elif d == HT - 2:
                        # per-chunk stores: this row's bytes are on the
                        # wire while the last row's matmuls run.
                        for ci, cs in enumerate(chunks[e]):
                            off = offs[e][ci]
                            psy = psB.tile([P, cs], F32, tag="psy")
                            b_group(e, d, ci, psy)
                            nc.scalar.copy(y_sb[:, off : off + cs], psy[:])
                            nc.sync.dma_start(
                                yt_d[e][:, d, off : off + cs],
                                y_sb[:, off : off + cs],
                            )
                    else:
                        # tail: big chunks first with per-chunk stores,
                        # then the small chunk halved across both queues
                        # (both copies precede any scalar dispatch).
                        order = list(range(len(chunks[e])))[::-1]
                        for ci in order[:-1]:
                            off, cs = offs[e][ci], chunks[e][ci]
                            psy = psB.tile([P, cs], F32, tag="psy")
                            b_group(e, d, ci, psy)
                            nc.scalar.copy(y_sb[:, off : off + cs], psy[:])
                            nc.sync.dma_start(
                                yt_d[e][:, d, off : off + cs],
                                y_sb[:, off : off + cs],
                            )
                        ci = order[-1]
                        off, cs = offs[e][ci], chunks[e][ci]
                        psy = psB.tile([P, cs], F32, tag="psy")
                        b_group(e, d, ci, psy)
                        ha = (cs // 2) // 4 * 4
                        nc.scalar.copy(y_sb[:, off : off + ha], psy[:, :ha])
                        nc.scalar.copy(
                            y_sb[:, off + ha : off + cs], psy[:, ha:cs]
                        )
                        nc.sync.dma_start(
                            yt_d[e][:, d, off : off + ha], y_sb[:, off : off + ha]
                        )
                        nc.scalar.dma_start(
                            yt_d[e][:, d, off + ha : off + cs],
                            y_sb[:, off + ha : off + cs],
                        )

            phase_b(0, last=False)

            # ---- e1: weights/tokens long resident; straight stream ----
            for h in range(HT):
                a_unit(1, h, range(len(chunks[1])))
            phase_b(1, last=True)

    _split_multi_waits(nc)
    return nc


def kernel(x, w_gate, w1, w2, wc):
    trace = bool(int(os.environ.get("BASS_MOE_TRACE", "0")))
    if trace:
        _install_profile_shim()

    import concourse.bass_utils as bass_utils

    bass_utils.upload_artifacts = lambda tmpdir: f"local://{tmpdir}"

    x = np.asarray(x, dtype=np.float32)
    w_gate = np.asarray(w_gate, dtype=np.float32)
    w1 = np.asarray(w1, dtype=np.float32)
    w2 = np.asarray(w2, dtype=np.float32)
    wc = np.asarray(wc, dtype=np.float32)

    b, s, d = x.shape
    xf = x.reshape(-1, d)
    n = xf.shape[0]

    # ---- Router on host (float64: stable ranking + gate values) ----
    logits = xf.astype(np.float64) @ w_gate.astype(np.float64)
    mx = logits.max(axis=1, keepdims=True)
    p = np.exp(logits - mx)
    p /= p.sum(axis=1, keepdims=True)
    top = np.argpartition(-logits, TOPK, axis=1)[:, :TOPK]  # top-2 ids (unordered)

    sel_tok = []  # per expert: token indices
    sel_gate = []  # per expert: gate values
    flat_e = top.ravel()
    flat_t = np.repeat(np.arange(n), TOPK)
    order = np.argsort(flat_e, kind="stable")
    se, st = flat_e[order], flat_t[order]
    bounds = np.searchsorted(se, np.arange(E + 1))
    counts = np.diff(bounds)
    for e in range(E):
        toks = st[bounds[e] : bounds[e + 1]]
        sel_tok.append(toks)
        sel_gate.append(p[toks, e])

    # ---- Slot assignment: biggest experts in slot 0, smallest in slot 1,
    # so each slot's uniform capacity hugs its experts' actual counts ----
    rank = np.argsort(-counts, kind="stable")
    slot_experts = [
        [int(rank[core + j * NCORES]) for j in range(EPC)] for core in range(NCORES)
    ]
    caps = []
    for j in range(EPC):
        cmax = max(counts[slot_experts[core][j]] for core in range(NCORES))
        caps.append(max(16, int(-(-cmax // 8) * 8)))

    # ---- Per-core input maps: bf16, pre-laid-out in SBUF tile order ----
    xf_bf = xf.astype(BF16)
    # weight layout [p, h, k, q] = w[k*128+p, h*128+q] (contiguous per
    # partition-row => 1 DMA descriptor per partition)
    def wlayout(w):
        return np.ascontiguousarray(
            w.astype(BF16).reshape(KT, P, HT, P).transpose(1, 2, 0, 3)
        )

    chunk_lists = [_token_chunks(caps[0]), _token_chunks(caps[1])]
    in_maps = []
    for core in range(NCORES):
        m = {}
        for j in range(EPC):
            e = slot_experts[core][j]
            C = caps[j]
            toks = sel_tok[e]
            # chunk-major: [chunk: [k: [c]]] flattened to [P, KT*C]
            xe = np.zeros((P, KT, C), dtype=BF16)
            xg = xf_bf[toks].T.reshape(KT, P, len(toks))
            xe[:, :, : len(toks)] = xg.transpose(1, 0, 2)
            xt = np.empty((P, KT * C), dtype=BF16)
            c0 = 0
            for cs in chunk_lists[j]:
                xt[:, c0 * KT : (c0 + cs) * KT] = xe[:, :, c0 : c0 + cs].reshape(
                    P, KT * cs
                )
                c0 += cs
            m[f"xt{j}"] = xt
            m[f"w1_{j}"] = wlayout(w1[e])
            m[f"w2_{j}"] = wlayout(w2[e])
            m[f"wc_{j}"] = wlayout(wc[e])
        in_maps.append(m)

    nc = _build_bass(caps)
    res = bass_utils.run_bass_kernel_spmd(
        nc, in_maps, list(range(NCORES)), trace=trace
    )
    if trace:
        kernel.last_exec_time_ns = res.exec_time_ns
        kernel.last_trace = (
            res.instructions_and_trace[1] if res.instructions_and_trace else None
        )

    # ---- Scatter-add back to token order, applying gates on host ----
    out = np.zeros((n, d), dtype=np.float64)
    for core in range(NCORES):
        for j in range(EPC):
            e = slot_experts[core][j]
            toks = sel_tok[e]
            yt = res.results[core][f"yt{j}"]  # [P, HT, C] bf16
            yv = (
                yt[:, :, : len(toks)]
                .transpose(1, 0, 2)
                .reshape(EMB, len(toks))
                .astype(np.float64)
            )
            out[toks] += sel_gate[e][:, None] * yv.T
    return out.astype(np.float32).reshape(b, s, d)
